# revision 1
# baseline (speedup 1.0000x reference)
"""Trainium2 Bass kernel for DicGaussianRBF.

out = concat([ones(N,1), data, exp(-5 * ||data - centers||^2)], axis=-1)
with data [65536, 256] f32, centers [2048, 256] f32 -> out [65536, 2305] f32.

Data-parallel over N across 8 NeuronCores; centers replicated. Per core:
  - centersT (bf16) built once via PE transpose; c2 row via ones-matmul of
    squared centersT.
  - per 128-row block: PE-transpose data tile, matmul psum = -2*x.c + c2
    (two 128-contraction bf16 matmuls + one rank-1), ScalarE activation
    exp(-5*psum + (-5*x2)) straight into the output tile; one 1.2 MB DMA out.
"""

import sys

for _p in ("/opt/trn_rl_repo",):
    if _p not in sys.path:
        sys.path.insert(0, _p)

import numpy as np

import concourse.bass as bass
import concourse.tile as tile
from concourse import bacc, mybir
from concourse import bass_utils
from concourse.masks import make_identity

N, D, K = 65536, 256, 2048
NCORES = 8
N_LOC = N // NCORES          # 8192 rows per core
OUT_W = 1 + D + K            # 2305
RB = N_LOC // 128            # 64 row blocks per core
S = 5.0

FP32 = mybir.dt.float32
BF16 = mybir.dt.bfloat16
Act = mybir.ActivationFunctionType

_cached_nc = None


def _build():
    nc = bacc.Bacc(
        "TRN2",
        target_bir_lowering=False,
        debug=False,
        enable_asserts=False,
        num_devices=NCORES,
    )
    data_ap = nc.dram_tensor("data", [N_LOC, D], FP32, kind="ExternalInput").ap()
    cent_ap = nc.dram_tensor("centers", [K, D], FP32, kind="ExternalInput").ap()
    out_ap = nc.dram_tensor("out", [N_LOC, OUT_W], FP32, kind="ExternalOutput").ap()

    with tile.TileContext(nc) as tc:
        with (
            tc.tile_pool(name="const", bufs=1) as const,
            tc.tile_pool(name="cload", bufs=2) as cload,
            tc.tile_pool(name="outp", bufs=4) as outp,
            tc.tile_pool(name="dtp", bufs=3) as dtp,
            tc.tile_pool(name="scr", bufs=2) as scr,
            tc.tile_pool(name="small", bufs=4) as small,
            tc.tile_pool(name="pstr", bufs=2, space="PSUM") as pstr,
            tc.tile_pool(name="psmm", bufs=5, space="PSUM") as psmm,
        ):
            ident = const.tile([128, 128], FP32)
            make_identity(nc, ident)
            ones_r1 = const.tile([1, 128], BF16)
            nc.vector.memset(ones_r1[:], 1.0)
            ones_col = const.tile([128, 1], BF16)
            nc.vector.memset(ones_col[:], 1.0)

            # centersT: [d-half][128, K] bf16, d-half 0 = dims 0:128, 1 = 128:256
            cT = [
                const.tile([128, K], BF16, name=f"cT{b}", tag=f"cT{b}")
                for b in range(2)
            ]
            sq = [
                const.tile([128, K], BF16, name=f"sq{b}", tag=f"sq{b}")
                for b in range(2)
            ]
            c2row = const.tile([1, K], BF16)

            for i in range(K // 128):
                ct = cload.tile([128, D], FP32)
                nc.sync.dma_start(ct[:], cent_ap[i * 128:(i + 1) * 128, :])
                pt = pstr.tile([128, 256], FP32, tag="pt")
                nc.tensor.transpose(pt[:, 0:128], ct[:, 0:128], ident[:])
                nc.tensor.transpose(pt[:, 128:256], ct[:, 128:256], ident[:])
                nc.vector.tensor_copy(cT[0][:, i * 128:(i + 1) * 128], pt[:, 0:128])
                nc.vector.tensor_copy(cT[1][:, i * 128:(i + 1) * 128], pt[:, 128:256])

            for b in range(2):
                nc.vector.tensor_mul(sq[b][:], cT[b][:], cT[b][:])

            for kb in range(4):
                ks = slice(kb * 512, (kb + 1) * 512)
                pc = psmm.tile([1, 512], FP32, tag="mm")
                nc.tensor.matmul(pc[:], ones_col[:], sq[0][:, ks], start=True, stop=False)
                nc.tensor.matmul(pc[:], ones_col[:], sq[1][:, ks], start=False, stop=True)
                nc.vector.tensor_copy(c2row[:, ks], pc[:])

            for rb in range(RB):
                rs = slice(rb * 128, (rb + 1) * 128)
                ot = outp.tile([128, OUT_W], FP32, tag="ot")
                nc.sync.dma_start(ot[:, 1:257], data_ap[rs, :])
                nc.gpsimd.memset(ot[:, 0:1], 1.0)

                sqd = scr.tile([128, D], FP32, tag="sqd")
                x2 = small.tile([128, 1], FP32, tag="x2")
                nc.scalar.activation(sqd[:], ot[:, 1:257], Act.Square, accum_out=x2[:])
                bias = small.tile([128, 1], FP32, tag="bias")
                nc.vector.tensor_scalar_mul(bias[:], x2[:], -S)

                pt = pstr.tile([128, 256], FP32, tag="pt")
                nc.tensor.transpose(pt[:, 0:128], ot[:, 1:129], ident[:])
                nc.tensor.transpose(pt[:, 128:256], ot[:, 129:257], ident[:])
                dT = dtp.tile([128, 256], BF16, tag="dT")
                nc.vector.tensor_scalar_mul(dT[:], pt[:], -2.0)

                for kb in range(4):
                    ks = slice(kb * 512, (kb + 1) * 512)
                    ps = psmm.tile([128, 512], FP32, tag="mm")
                    nc.tensor.matmul(ps[:], dT[:, 0:128], cT[0][:, ks], start=True, stop=False)
                    nc.tensor.matmul(ps[:], dT[:, 128:256], cT[1][:, ks], start=False, stop=False)
                    nc.tensor.matmul(ps[:], ones_r1[:], c2row[:, ks], start=False, stop=True)
                    nc.scalar.activation(
                        ot[:, 257 + kb * 512: 257 + (kb + 1) * 512],
                        ps[:],
                        Act.Exp,
                        bias=bias[:],
                        scale=-S,
                    )

                nc.sync.dma_start(out_ap[rs, :], ot[:])

    nc.compile()
    return nc


def _get_nc():
    global _cached_nc
    if _cached_nc is None:
        _cached_nc = _build()
    return _cached_nc


def kernel(data, centers):
    data = np.ascontiguousarray(np.asarray(data, dtype=np.float32))
    centers = np.ascontiguousarray(np.asarray(centers, dtype=np.float32))
    assert data.shape == (N, D) and centers.shape == (K, D)

    nc = _get_nc()
    in_maps = [
        {"data": data[i * N_LOC:(i + 1) * N_LOC], "centers": centers}
        for i in range(NCORES)
    ]
    res = bass_utils.run_bass_kernel_spmd(nc, in_maps, core_ids=list(range(NCORES)))
    return np.concatenate([res.results[i]["out"] for i in range(NCORES)], axis=0)


# revision 2
# speedup vs baseline: 2.1289x; 2.1289x over previous
"""Trainium2 Bass kernel for DicGaussianRBF.

out = concat([ones(N,1), data, exp(-5 * ||data - centers||^2)], axis=-1)
with data [65536, 256] f32, centers [2048, 256] f32 -> out [65536, 2305] f32.

Data-parallel over N across 8 NeuronCores; centers replicated. Per core
(8192 rows, 64 row-blocks of 128):

  setup: centersT (bf16) via PE transpose; c2 row via ones-matmul of the
  squared centersT.

  steady state (software-skewed by 2 row-blocks so the PE matmul stream
  never waits on the transpose->DVE-cast chain):
    - input staged 8 row-blocks (1 MB) per SWDGE DMA into `din` tiles that
      also carry the ones column -> the [*, 0:257] output block is DMA'd
      straight from din.
    - per row-block: DVE computes bias = -5*||x||^2 in one
      scalar_tensor_tensor; PE transposes the data tile; DVE casts it to
      bf16 with a -2 scale; 12 matmuls accumulate psum = -2 x.c + c2 into
      two 2-bank psum tiles; ScalarE evaluates exp(-5*psum + bias) at
      N=1024 straight into the rbf output tile; HWDGE DMAs it out.
"""

import sys

for _p in ("/opt/trn_rl_repo",):
    if _p not in sys.path:
        sys.path.insert(0, _p)

import numpy as np

import concourse.bass as bass
import concourse.tile as tile
from concourse import bacc, mybir
from concourse import bass_utils
from concourse.masks import make_identity

N, D, K = 65536, 256, 2048
NCORES = 8
N_LOC = N // NCORES          # 8192 rows per core
OUT_W = 1 + D + K            # 2305
RB = N_LOC // 128            # 64 row blocks per core
SB = 8                       # row blocks per input staging DMA
PRE = 2                      # transpose pipeline lookahead (row blocks)
S = 5.0

FP32 = mybir.dt.float32
BF16 = mybir.dt.bfloat16
Act = mybir.ActivationFunctionType
MULT = mybir.AluOpType.mult

_cached_nc = None


def _build():
    nc = bacc.Bacc(
        "TRN2",
        target_bir_lowering=False,
        debug=False,
        enable_asserts=False,
        num_devices=NCORES,
    )
    data_ap = nc.dram_tensor("data", [N_LOC, D], FP32, kind="ExternalInput").ap()
    cent_ap = nc.dram_tensor("centers", [K, D], FP32, kind="ExternalInput").ap()
    out_ap = nc.dram_tensor("out", [N_LOC, OUT_W], FP32, kind="ExternalOutput").ap()

    with tile.TileContext(nc) as tc:
        with (
            tc.tile_pool(name="const", bufs=1) as const,
            tc.tile_pool(name="cload", bufs=2) as cload,
            tc.tile_pool(name="dinp", bufs=3) as dinp,
            tc.tile_pool(name="rbfp", bufs=4) as rbfp,
            tc.tile_pool(name="dtp", bufs=4) as dtp,
            tc.tile_pool(name="scrp", bufs=2) as scrp,
            tc.tile_pool(name="biasp", bufs=6) as biasp,
            tc.tile_pool(name="pstr", bufs=2, space="PSUM") as pstr,
            tc.tile_pool(name="psmm", bufs=3, space="PSUM") as psmm,
        ):
            ident = const.tile([128, 128], FP32)
            make_identity(nc, ident)
            ones_r1 = const.tile([1, 128], BF16)
            nc.vector.memset(ones_r1[:], 1.0)
            ones_col = const.tile([128, 1], BF16)
            nc.vector.memset(ones_col[:], 1.0)

            # centersT: [d-half][128, K] bf16; d-half 0 = dims 0:128, 1 = 128:256
            cT = [
                const.tile([128, K], BF16, name=f"cT{b}", tag=f"cT{b}")
                for b in range(2)
            ]
            sq = [
                const.tile([128, K], BF16, name=f"sq{b}", tag=f"sq{b}")
                for b in range(2)
            ]
            c2row = const.tile([1, K], BF16)

            for i in range(K // 128):
                ct = cload.tile([128, D], FP32, tag="ct")
                nc.gpsimd.dma_start(ct[:], cent_ap[i * 128:(i + 1) * 128, :])
                pt = pstr.tile([128, 256], FP32, tag="pt")
                nc.tensor.transpose(pt[:, 0:128], ct[:, 0:128], ident[:])
                nc.tensor.transpose(pt[:, 128:256], ct[:, 128:256], ident[:])
                nc.vector.tensor_copy(cT[0][:, i * 128:(i + 1) * 128], pt[:, 0:128])
                nc.vector.tensor_copy(cT[1][:, i * 128:(i + 1) * 128], pt[:, 128:256])

            for b in range(2):
                nc.vector.tensor_mul(sq[b][:], cT[b][:], cT[b][:])

            for kb in range(4):
                ks = slice(kb * 512, (kb + 1) * 512)
                pc = pstr.tile([1, 512], FP32, tag="pt", name="pc")
                nc.tensor.matmul(pc[:], ones_col[:], sq[0][:, ks], start=True, stop=False)
                nc.tensor.matmul(pc[:], ones_col[:], sq[1][:, ks], start=False, stop=True)
                nc.vector.tensor_copy(c2row[:, ks], pc[:])

            din_tiles = {}
            stage = {}
            for step in range(RB + PRE):
                # ---- front of the pipe: stage input, bias, transpose, cast
                rb = step
                if rb < RB:
                    if rb % SB == 0:
                        sb = rb // SB
                        din = dinp.tile([128, SB * 257], FP32, tag="din")
                        din_tiles[sb] = din
                        ones_view = din[:].rearrange("p (r c) -> p r c", c=257)[:, :, 0:1]
                        nc.gpsimd.memset(ones_view, 1.0)
                        dst = din[:].rearrange("p (r c) -> p r c", c=257)[:, :, 1:257]
                        src = data_ap[sb * SB * 128:(sb + 1) * SB * 128, :].rearrange(
                            "(r p) d -> p r d", p=128
                        )
                        nc.gpsimd.dma_start(dst, src)
                    din = din_tiles[rb // SB]
                    b = rb % SB
                    dcol = din[:, b * 257 + 1: b * 257 + 257]

                    scratch = scrp.tile([128, D], BF16, tag="scr")
                    bias = biasp.tile([128, 1], FP32, tag="bias")
                    nc.vector.scalar_tensor_tensor(
                        scratch[:], dcol, -S, dcol, MULT, MULT, accum_out=bias[:]
                    )

                    pt = pstr.tile([128, 256], FP32, tag="pt")
                    nc.tensor.transpose(pt[:, 0:128], dcol[:, 0:128], ident[:])
                    nc.tensor.transpose(pt[:, 128:256], dcol[:, 128:256], ident[:])
                    dT = dtp.tile([128, 256], BF16, tag="dT")
                    nc.vector.tensor_scalar_mul(dT[:], pt[:], -2.0)
                    stage[rb] = (dT, bias, din[:, b * 257:(b + 1) * 257])

                # ---- back of the pipe: matmuls, exp, output DMA
                rbm = step - PRE
                if rbm >= 0:
                    dT, bias, din_slice = stage.pop(rbm)
                    rs = slice(rbm * 128, (rbm + 1) * 128)
                    ot = rbfp.tile([128, K], FP32, tag="ot")
                    for half in range(2):
                        ks0 = slice((2 * half) * 512, (2 * half + 1) * 512)
                        ks1 = slice((2 * half + 1) * 512, (2 * half + 2) * 512)
                        ps = psmm.tile([128, 1024], FP32, tag="mm")
                        h0 = ps[:, 0:512]
                        h1 = ps[:, 512:1024]
                        nc.tensor.matmul(h0, dT[:, 0:128], cT[0][:, ks0], start=True, stop=False)
                        nc.tensor.matmul(h1, dT[:, 0:128], cT[0][:, ks1], start=True, stop=False)
                        nc.tensor.matmul(h0, dT[:, 128:256], cT[1][:, ks0], start=False, stop=False)
                        nc.tensor.matmul(h1, dT[:, 128:256], cT[1][:, ks1], start=False, stop=False)
                        nc.tensor.matmul(h0, ones_r1[:], c2row[:, ks0], start=False, stop=True)
                        nc.tensor.matmul(h1, ones_r1[:], c2row[:, ks1], start=False, stop=True)
                        nc.scalar.activation(
                            ot[:, half * 1024:(half + 1) * 1024],
                            ps[:],
                            Act.Exp,
                            bias=bias[:],
                            scale=-S,
                        )
                    nc.sync.dma_start(out_ap[rs, 257:OUT_W], ot[:])
                    nc.sync.dma_start(out_ap[rs, 0:257], din_slice)

    nc.compile()
    return nc


def _get_nc():
    global _cached_nc
    if _cached_nc is None:
        _cached_nc = _build()
    return _cached_nc


def kernel(data, centers):
    data = np.ascontiguousarray(np.asarray(data, dtype=np.float32))
    centers = np.ascontiguousarray(np.asarray(centers, dtype=np.float32))
    assert data.shape == (N, D) and centers.shape == (K, D)

    nc = _get_nc()
    in_maps = [
        {"data": data[i * N_LOC:(i + 1) * N_LOC], "centers": centers}
        for i in range(NCORES)
    ]
    res = bass_utils.run_bass_kernel_spmd(nc, in_maps, core_ids=list(range(NCORES)))
    return np.concatenate([res.results[i]["out"] for i in range(NCORES)], axis=0)


# revision 8
# speedup vs baseline: 2.2886x; 1.0750x over previous
"""Trainium2 Bass kernel for DicGaussianRBF.

out = concat([ones(N,1), data, exp(-5 * ||data - centers||^2)], axis=-1)
with data [65536, 256] f32, centers [2048, 256] f32 -> out [65536, 2305] f32.

Data-parallel over N across 8 NeuronCores; centers replicated. Per core
(8192 rows, 64 row-blocks of 128):

  setup: centersT (bf16) via PE transpose; c2 row via ones-matmul of the
  squared centersT.

  steady state (software-skewed by 2 row-blocks so the PE matmul stream
  never waits on the transpose->DVE-cast chain):
    - input staged 8 row-blocks (1 MB) per SWDGE DMA into `din` tiles that
      also carry the ones column -> the [*, 0:257] output block is DMA'd
      straight from din.
    - per row-block: DVE computes bias = -5*||x||^2 in one
      scalar_tensor_tensor; PE transposes the data tile; DVE casts it to
      bf16 with a -2 scale; 12 matmuls accumulate psum = -2 x.c + c2 into
      two 2-bank psum tiles; ScalarE evaluates exp(-5*psum + bias) at
      N=1024 straight into the rbf output tile; HWDGE DMAs it out.
"""

import sys

for _p in ("/opt/trn_rl_repo",):
    if _p not in sys.path:
        sys.path.insert(0, _p)

import numpy as np

import concourse.bass as bass
import concourse.tile as tile
from concourse import bacc, mybir
from concourse import bass_utils
from concourse.masks import make_identity

N, D, K = 65536, 256, 2048
NCORES = 8
N_LOC = N // NCORES          # 8192 rows per core
OUT_W = 1 + D + K            # 2305
RB = N_LOC // 128            # 64 row blocks per core
SB = 8                       # row blocks per input staging DMA
PRE = 2                      # transpose pipeline lookahead (row blocks)
S = 5.0

FP32 = mybir.dt.float32
BF16 = mybir.dt.bfloat16
Act = mybir.ActivationFunctionType
MULT = mybir.AluOpType.mult

_cached_nc = None


def _build():
    nc = bacc.Bacc(
        "TRN2",
        target_bir_lowering=False,
        debug=False,
        enable_asserts=False,
        num_devices=NCORES,
    )
    data_ap = nc.dram_tensor("data", [N_LOC, D], FP32, kind="ExternalInput").ap()
    cent_ap = nc.dram_tensor("centers", [K, D], FP32, kind="ExternalInput").ap()
    out_ap = nc.dram_tensor("out", [N_LOC, OUT_W], FP32, kind="ExternalOutput").ap()

    with tile.TileContext(nc) as tc:
        with (
            tc.tile_pool(name="const", bufs=1) as const,
            tc.tile_pool(name="cload", bufs=1) as cload,
            tc.tile_pool(name="dinp", bufs=4) as dinp,
            tc.tile_pool(name="rbfp", bufs=4) as rbfp,
            tc.tile_pool(name="dtp", bufs=4) as dtp,
            tc.tile_pool(name="scrp", bufs=2) as scrp,
            tc.tile_pool(name="biasp", bufs=6) as biasp,
            tc.tile_pool(name="pstr", bufs=2, space="PSUM") as pstr,
            tc.tile_pool(name="psmm", bufs=3, space="PSUM") as psmm,
        ):
            ident = const.tile([128, 128], FP32)
            make_identity(nc, ident)
            ones_r1 = const.tile([1, 128], BF16)
            nc.vector.memset(ones_r1[:], 1.0)
            ones_col = const.tile([128, 1], BF16)
            nc.vector.memset(ones_col[:], 1.0)

            # centersT: [d-half][128, K] bf16; d-half 0 = dims 0:128, 1 = 128:256
            cT = [
                const.tile([128, K], BF16, name=f"cT{b}", tag=f"cT{b}")
                for b in range(2)
            ]
            sq = [
                const.tile([128, K], BF16, name=f"sq{b}", tag=f"sq{b}")
                for b in range(2)
            ]
            c2row = const.tile([1, K], BF16)

            din_tiles = {}

            def load_super_block(sb):
                din = dinp.tile([128, SB * 257], FP32, tag="din", name="din")
                din_tiles[sb] = din
                din3 = din[:].rearrange("p (r c) -> p r c", c=257)
                nc.gpsimd.memset(din3[:, :, 0:1], 1.0)
                src = data_ap[sb * SB * 128:(sb + 1) * SB * 128, :].rearrange(
                    "(r p) d -> p r d", p=128
                )
                nc.gpsimd.dma_start(din3[:, :, 1:257], src)
                # ones+data block of the output comes straight from din
                dst = out_ap[sb * SB * 128:(sb + 1) * SB * 128, 0:257].rearrange(
                    "(r p) c -> p r c", p=128
                )
                nc.sync.dma_start(dst, din3[:, :, :])

            # prefetch the first data super-block before the centers load so
            # the transpose pipeline can start while centers are processed
            load_super_block(0)

            call = cload.tile([128, (K // 128) * D], FP32)
            csrc = cent_ap[:, :].rearrange("(t p) d -> p t d", p=128)
            nc.gpsimd.dma_start(
                call[:].rearrange("p (t d) -> p t d", d=D), csrc
            )
            load_super_block(1)

            for i in range(K // 128):
                ct = call[:, i * D:(i + 1) * D]
                pt = pstr.tile([128, 256], FP32, tag="pt")
                nc.tensor.transpose(pt[:, 0:128], ct[:, 0:128], ident[:])
                nc.tensor.transpose(pt[:, 128:256], ct[:, 128:256], ident[:])
                nc.vector.tensor_copy(cT[0][:, i * 128:(i + 1) * 128], pt[:, 0:128])
                nc.vector.tensor_copy(cT[1][:, i * 128:(i + 1) * 128], pt[:, 128:256])

            for b in range(2):
                nc.vector.tensor_mul(sq[b][:], cT[b][:], cT[b][:])

            for kb in range(4):
                ks = slice(kb * 512, (kb + 1) * 512)
                pc = pstr.tile([1, 512], FP32, tag="pt", name="pc")
                nc.tensor.matmul(pc[:], ones_col[:], sq[0][:, ks], start=True, stop=False)
                nc.tensor.matmul(pc[:], ones_col[:], sq[1][:, ks], start=False, stop=True)
                nc.vector.tensor_copy(c2row[:, ks], pc[:])

            stage = {}
            for step in range(RB + PRE):
                # ---- front of the pipe: stage input, bias, transpose, cast
                rb = step
                if rb < RB:
                    if rb % SB == 0 and rb // SB + 2 < RB // SB:
                        load_super_block(rb // SB + 2)
                    din = din_tiles[rb // SB]
                    b = rb % SB
                    dcol = din[:, b * 257 + 1: b * 257 + 257]

                    scratch = scrp.tile([128, D], BF16, tag="scr")
                    bias = biasp.tile([128, 1], FP32, tag="bias")
                    nc.vector.scalar_tensor_tensor(
                        scratch[:], dcol, -S, dcol, MULT, MULT, accum_out=bias[:]
                    )

                    pt = pstr.tile([128, 256], FP32, tag="pt")
                    nc.tensor.transpose(pt[:, 0:128], dcol[:, 0:128], ident[:])
                    nc.tensor.transpose(pt[:, 128:256], dcol[:, 128:256], ident[:])
                    dT = dtp.tile([128, 256], BF16, tag="dT")
                    nc.vector.tensor_scalar_mul(dT[:], pt[:], -2.0)
                    stage[rb] = (dT, bias)

                # ---- back of the pipe: matmuls, exp, output DMA
                rbm = step - PRE
                if rbm >= 0:
                    dT, bias = stage.pop(rbm)
                    rs = slice(rbm * 128, (rbm + 1) * 128)
                    ot = rbfp.tile([128, K], FP32, tag="ot")
                    for half in range(2):
                        ks0 = slice((2 * half) * 512, (2 * half + 1) * 512)
                        ks1 = slice((2 * half + 1) * 512, (2 * half + 2) * 512)
                        ps = psmm.tile([128, 1024], FP32, tag="mm")
                        h0 = ps[:, 0:512]
                        h1 = ps[:, 512:1024]
                        nc.tensor.matmul(h0, dT[:, 0:128], cT[0][:, ks0], start=True, stop=False)
                        nc.tensor.matmul(h1, dT[:, 0:128], cT[0][:, ks1], start=True, stop=False)
                        nc.tensor.matmul(h0, dT[:, 128:256], cT[1][:, ks0], start=False, stop=False)
                        nc.tensor.matmul(h1, dT[:, 128:256], cT[1][:, ks1], start=False, stop=False)
                        nc.tensor.matmul(h0, ones_r1[:], c2row[:, ks0], start=False, stop=True)
                        nc.tensor.matmul(h1, ones_r1[:], c2row[:, ks1], start=False, stop=True)
                        nc.scalar.activation(
                            ot[:, half * 1024:(half + 1) * 1024],
                            ps[:],
                            Act.Exp,
                            bias=bias[:],
                            scale=-S,
                        )
                    nc.sync.dma_start(out_ap[rs, 257:OUT_W], ot[:])

    nc.compile()
    return nc


def _get_nc():
    global _cached_nc
    if _cached_nc is None:
        _cached_nc = _build()
    return _cached_nc


def kernel(data, centers):
    data = np.ascontiguousarray(np.asarray(data, dtype=np.float32))
    centers = np.ascontiguousarray(np.asarray(centers, dtype=np.float32))
    assert data.shape == (N, D) and centers.shape == (K, D)

    nc = _get_nc()
    in_maps = [
        {"data": data[i * N_LOC:(i + 1) * N_LOC], "centers": centers}
        for i in range(NCORES)
    ]
    res = bass_utils.run_bass_kernel_spmd(nc, in_maps, core_ids=list(range(NCORES)))
    return np.concatenate([res.results[i]["out"] for i in range(NCORES)], axis=0)
